# revision 28
# baseline (speedup 1.0000x reference)
"""Trainium2 Bass kernel for nn_AttentionBase (channel attention with conv qkv).

Math restructuring (validated in fp64/fp32 numpy vs the jax reference):
  - conv1 (1x1) folds into conv2 (k=3): C_k = W2[:,:,k] @ W1  -> one k=3 conv.
  - The per-head 16x16 channel-attention matrix A only needs Gram stats of the
    UN-normalized q,k:  G_qk = q @ k^T  and per-channel sumsq of q and k
    (L2 normalization and `scale` fold into a rank-1 rescale of G_qk).
  - v is never materialized:  out = Wp @ BlockDiag(A) @ v = conv(x, M @ V_k)
    with M = Wp @ BlockDiag(A) computed on-device (tiny matmuls).

Per core (1 batch element per core, 8 cores):
  pass 1: token-major k=3 conv (bf16) -> q,k tiles [128t x 256c]; accumulate
          G_qk (PE), sumsq via ones-vector matmul of squared tiles (PE).
  epilogue: norms via exp(-0.5*ln(ss)), rank-1 rescale (K=1 outer-product
          matmul), per-head softmax, M^T and folded pass-2 weights.
  pass 2: k=3 conv of x (fp32r) with folded weights -> output.
"""

import os
import sys

import numpy as np

sys.path.insert(0, "/opt/trn_rl_repo")

import ml_dtypes  # noqa: E402

import concourse.bass as bass  # noqa: E402
import concourse.tile as tile  # noqa: E402
from concourse import bacc, mybir  # noqa: E402
from concourse.bass_utils import run_bass_kernel_spmd  # noqa: E402

F32 = mybir.dt.float32
F32R = mybir.dt.float32r
BF16 = mybir.dt.bfloat16

B, C, N = 8, 128, 16384
HEADS, HD = 8, 16
NCORES = 8
CHUNK = 1024          # DMA / bf16-convert chunk (tokens)
T1 = 128              # pass-1 token tile
T2 = 512              # pass-2 token tile
AFT = mybir.ActivationFunctionType


def build_program():
    nc = bacc.Bacc(None, target_bir_lowering=False)

    x_d = nc.dram_tensor("x", [C, N], F32, kind="ExternalInput")
    wqk_d = nc.dram_tensor("wqk", [C, 3 * 256], BF16, kind="ExternalInput")
    wv_d = nc.dram_tensor("wv", [C, 3 * C], F32, kind="ExternalInput")
    wpt_d = nc.dram_tensor("wpt", [C, C], F32, kind="ExternalInput")
    svec_d = nc.dram_tensor("svec", [1, C], F32, kind="ExternalInput")
    mask_d = nc.dram_tensor("mask", [C, C], F32, kind="ExternalInput")
    out_d = nc.dram_tensor("out", [C, N], F32, kind="ExternalOutput")

    with tile.TileContext(nc) as tc:
        with (
            tc.tile_pool(name="const", bufs=1) as const,
            tc.tile_pool(name="xpool", bufs=1) as xpool,
            tc.tile_pool(name="work", bufs=3) as work,
            tc.tile_pool(name="epi", bufs=1) as epi,
            tc.tile_pool(name="psum", bufs=1, space="PSUM") as psum,
        ):
            # ---- constants ----
            wqk_sb = const.tile([C, 3 * 256], BF16)
            nc.sync.dma_start(out=wqk_sb, in_=wqk_d[:, :])
            wv_sb = const.tile([C, 3 * C], F32)
            nc.sync.dma_start(out=wv_sb, in_=wv_d[:, :])
            wpt_sb = const.tile([C, C], F32)
            nc.sync.dma_start(out=wpt_sb, in_=wpt_d[:, :])
            svec_sb = const.tile([1, C], F32)
            nc.sync.dma_start(out=svec_sb, in_=svec_d[:, :])
            mask_sb = const.tile([C, C], F32)
            nc.sync.dma_start(out=mask_sb, in_=mask_d[:, :])
            ones_sb = const.tile([C, 1], BF16)
            nc.vector.memset(ones_sb, 1.0)

            # warm the ACT ln/exp table early so the epilogue doesn't stall
            warm_sb = const.tile([1, 1], F32)
            nc.vector.memset(warm_sb, 1.0)
            nc.scalar.activation(warm_sb, warm_sb, AFT.Exp)

            # ---- x resident in SBUF: fp32r (pass 2) + bf16 (pass 1) ----
            # column j of x_sb corresponds to x[:, j-1]; cols 0 and N+1 are the
            # zero padding of the k=3 convs. fp32r requires a rounding producer
            # (ACT copy), so DMA lands in fp32 staging chunks first.
            x_sb = xpool.tile([C, N + 2], F32R)
            xbf_sb = xpool.tile([C, N + 2], BF16)
            nc.vector.memset(x_sb[:, 0:1].bitcast(F32), 0.0)
            nc.vector.memset(x_sb[:, N + 1 : N + 2].bitcast(F32), 0.0)
            nc.vector.memset(xbf_sb[:, 0:1], 0.0)
            nc.vector.memset(xbf_sb[:, N + 1 : N + 2], 0.0)
            for ci in range(N // CHUNK):
                lo = 1 + ci * CHUNK
                stg = work.tile([C, CHUNK], F32, tag="stg")
                nc.sync.dma_start(
                    out=stg, in_=x_d[:, ci * CHUNK : (ci + 1) * CHUNK]
                )
                nc.scalar.copy(out=x_sb[:, lo : lo + CHUNK], in_=stg)
                nc.vector.tensor_copy(
                    out=xbf_sb[:, lo : lo + CHUNK], in_=stg
                )

            # ---- pass 1: conv (token-major) + Gram accumulation ----
            # Two token-tiles (A at cols 0:256, B at 256:512 of one PSUM bank)
            # share one DVE cast + one ACT square. Gram matmuls run one batch
            # behind (software pipelining) so the in-order PE never waits on
            # the DVE/ACT of the same batch.
            HB = 2   # token-tiles per batch (one cast/square per batch)
            LAG = 3  # gram matmuls run LAG batches behind the convs
            gqk_ps = psum.tile([C, C], F32, tag="gqk")
            gss_ps = psum.tile([1, HB * 2 * C], F32, tag="gss")
            nb = N // (HB * T1)
            qk_hist = {}
            for b in range(nb + LAG):
                if b < nb:
                    qk_ps = psum.tile([T1, HB * 2 * C], F32, tag="qk", bufs=4)
                    for half in range(HB):
                        t = HB * b + half
                        for k in range(3):
                            o = t * T1 + k
                            nc.tensor.matmul(
                                qk_ps[:, half * 256 : (half + 1) * 256],
                                lhsT=xbf_sb[:, o : o + T1],
                                rhs=wqk_sb[:, k * 256 : (k + 1) * 256],
                                start=(k == 0),
                                stop=(k == 2),
                            )
                    qk_sb = work.tile([T1, HB * 2 * C], BF16, tag="qk_sb", bufs=6)
                    nc.vector.tensor_copy(out=qk_sb, in_=qk_ps)
                    qksq_sb = work.tile([T1, HB * 2 * C], BF16, tag="qksq_sb", bufs=6)
                    nc.scalar.square(qksq_sb, qk_sb)
                    qk_hist[b] = (qk_sb, qksq_sb)
                if b >= LAG:
                    pb = b - LAG
                    pqk, psq = qk_hist.pop(pb)
                    for half in range(HB):
                        nc.tensor.matmul(
                            gqk_ps,
                            lhsT=pqk[:, half * 256 : half * 256 + C],
                            rhs=pqk[:, half * 256 + C : half * 256 + 2 * C],
                            start=(pb == 0 and half == 0),
                            stop=(pb == nb - 1 and half == HB - 1),
                        )
                    for gh in range(HB // 2):
                        nc.tensor.matmul(
                            gss_ps[:, gh * 512 : (gh + 1) * 512],
                            lhsT=ones_sb,
                            rhs=psq[:, gh * 512 : (gh + 1) * 512],
                            start=(pb == 0),
                            stop=(pb == nb - 1),
                        )

            # ---- epilogue: attention matrix + folded pass-2 weights ----
            ss2_sb = epi.tile([1, HB * 2 * C], F32)
            nc.vector.tensor_copy(out=ss2_sb, in_=gss_ps)
            ss_sb = epi.tile([1, 2 * C], F32)
            nc.vector.tensor_add(
                ss_sb, ss2_sb[:, 0 : 2 * C], ss2_sb[:, 2 * C : 4 * C]
            )
            for blk in range(2, HB):
                nc.vector.tensor_add(
                    ss_sb, ss_sb, ss2_sb[:, blk * 2 * C : (blk + 1) * 2 * C]
                )
            # r = 1/max(sqrt(ss), 1e-12) == rsqrt(max(ss, 1e-24)), via ln/exp
            # (single ACT table set; Rsqrt activation is banned for accuracy).
            nc.vector.tensor_scalar_max(ss_sb, ss_sb, 1e-24)
            nc.scalar.activation(ss_sb, ss_sb, AFT.Ln)
            r_sb = epi.tile([1, 2 * C], F32)
            nc.scalar.activation(r_sb, ss_sb, AFT.Exp, scale=-0.5)
            rq_sb = epi.tile([1, C], F32)
            nc.vector.tensor_mul(rq_sb, r_sb[:, 0:C], svec_sb)

            outer_ps = psum.tile([C, C], F32, tag="epi", bufs=1)
            nc.tensor.matmul(outer_ps, lhsT=rq_sb, rhs=r_sb[:, C : 2 * C])
            outer_sb = epi.tile([C, C], F32)
            nc.vector.tensor_copy(out=outer_sb, in_=outer_ps)

            # A = softmax over each 16x16 diagonal block; the additive mask is
            # -1e30 off-block, so exp underflows to exactly 0 there — giving
            # the BlockDiag(A) layout the M^T matmul needs with full-width ops
            # (engine partition bases must be 32-aligned; 16-row slices aren't).
            a_sb = epi.tile([C, C], F32)
            nc.vector.tensor_mul(a_sb, gqk_ps, outer_sb)
            nc.vector.tensor_add(a_sb, a_sb, mask_sb)
            negmax = epi.tile([C, 1], F32)
            rsum = epi.tile([C, 1], F32)
            nc.vector.reduce_max(
                out=negmax, in_=a_sb, axis=mybir.AxisListType.X, negate=True
            )
            nc.scalar.activation(a_sb, a_sb, AFT.Exp, bias=negmax)
            nc.vector.reduce_sum(out=rsum, in_=a_sb, axis=mybir.AxisListType.X)
            nc.vector.reciprocal(rsum, rsum)
            nc.vector.tensor_scalar_mul(a_sb, a_sb, rsum)

            # MT[d, m] = sum_c A[c, d] * WpT[c, m]
            mt_ps = psum.tile([C, C], F32, tag="epi", bufs=1)
            nc.tensor.matmul(mt_ps, lhsT=a_sb, rhs=wpt_sb)
            mt_sb = epi.tile([C, C], F32)
            nc.vector.tensor_copy(out=mt_sb, in_=mt_ps)

            foldT_sb = epi.tile([C, 3 * C], F32R)
            for k in range(3):
                fold_ps = psum.tile([C, C], F32, tag="epi", bufs=1)
                nc.tensor.matmul(
                    fold_ps, lhsT=wv_sb[:, k * C : (k + 1) * C], rhs=mt_sb
                )
                nc.vector.tensor_copy(
                    out=foldT_sb[:, k * C : (k + 1) * C], in_=fold_ps
                )

            # ---- pass 2: folded k=3 conv of x (fp32r), channel-major ----
            for j in range(N // T2):
                o_ps = psum.tile([C, T2], F32, tag="qk", bufs=4)
                for k in range(3):
                    o = j * T2 + k
                    nc.tensor.matmul(
                        o_ps,
                        lhsT=foldT_sb[:, k * C : (k + 1) * C],
                        rhs=x_sb[:, o : o + T2],
                        start=(k == 0),
                        stop=(k == 2),
                    )
                o_sb = work.tile([C, T2], F32, tag="o_sb")
                if j % 2 == 0:
                    nc.scalar.copy(out=o_sb, in_=o_ps)
                else:
                    nc.vector.tensor_copy(out=o_sb, in_=o_ps)
                nc.sync.dma_start(
                    out=out_d[:, j * T2 : (j + 1) * T2], in_=o_sb
                )

    nc.finalize()
    return nc


def _prep_weights(w_qkv1, w_qkv2, w_proj, scale):
    W1 = np.asarray(w_qkv1, np.float32)[:, :, 0]          # [384, 128]
    W2 = np.asarray(w_qkv2, np.float32)                   # [384, 384, 3]
    Ck = np.stack([W2[:, :, k] @ W1 for k in range(3)])   # [3, 384, 128]
    Qk, Kk, Vk = Ck[:, 0:C, :], Ck[:, C : 2 * C, :], Ck[:, 2 * C :, :]
    wqk = np.concatenate(
        [np.concatenate([Qk[k].T, Kk[k].T], axis=1) for k in range(3)], axis=1
    )                                                     # [128, 3*256]
    wv = np.concatenate([Vk[k] for k in range(3)], axis=1)  # [128, 3*128]
    wpt = np.ascontiguousarray(np.asarray(w_proj, np.float32)[:, :, 0].T)
    svec = np.repeat(np.asarray(scale, np.float32)[:, 0, 0], HD)[None, :]
    mask = np.full((C, C), -1e30, np.float32)
    for h in range(HEADS):
        mask[h * HD : (h + 1) * HD, h * HD : (h + 1) * HD] = 0.0
    return (
        wqk.astype(ml_dtypes.bfloat16),
        np.ascontiguousarray(wv, np.float32),
        wpt,
        np.ascontiguousarray(svec, np.float32),
        mask,
    )


_CACHE = {}


def kernel(x, w_qkv1, w_qkv2, w_proj, scale, _trace=False, _tmpdir=None):
    x = np.asarray(x, np.float32)
    assert x.shape == (B, C, N), x.shape
    wqk, wv, wpt, svec, mask = _prep_weights(w_qkv1, w_qkv2, w_proj, scale)

    if "nc" not in _CACHE:
        _CACHE["nc"] = build_program()
    nc = _CACHE["nc"]

    in_maps = [
        {
            "x": np.ascontiguousarray(x[i]),
            "wqk": wqk,
            "wv": wv,
            "wpt": wpt,
            "svec": svec,
            "mask": mask,
        }
        for i in range(NCORES)
    ]
    res = run_bass_kernel_spmd(
        nc,
        in_maps,
        core_ids=list(range(NCORES)),
        trace=_trace,
        tmpdir=_tmpdir,
    )
    out = np.stack([r["out"] for r in res.results]).astype(np.float32)
    if _trace:
        _CACHE["last_result"] = res
    return out
